# revision 2
# baseline (speedup 1.0000x reference)
"""Trainium2 Bass kernel for nn_CrossAttention_31078383354530.

Reference computation (b=2, n=m=2048, qd=1024, cd=768, heads=8, dh=128):
    q = x @ Wq; k = ctx @ Wk; v = ctx @ Wv  (split into 8 heads of 128)
    sim = (q @ k^T) * dh**-0.5 over the FLATTENED (b*n)=4096 token axis
    sim = (sim - mean)*1.5 + mean;  attn = softmax(sim)   [the mean-centering
        is a per-row constant shift, so softmax((sim-mu)*tau+mu) ==
        softmax(tau*scale*(q@k^T)) exactly]
    out = attn @ v -> merge heads -> y = out @ Wout + bout

Sharding (8 cores): context-token-sharded K/V projection + AllGather of the
bf16 K/V (all heads), then each core runs all 8 heads' attention for its own
512-query-token slice and its own final projection -> output is a disjoint
row-slice per core (no reduction needed on host).

All matmuls run at 1 cycle/row: f32 data uses the float32r PE mode for the
projections, and bf16 operands for sim / attn@v / rowsum / final proj.
Softmax rowsum is computed with an ones-stationary matmul (M=1) fused into
the PV stream; normalization uses nc.vector.reciprocal + gpsimd
partition_broadcast + one tensor_tensor multiply per head.
"""

import sys

if "/opt/trn_rl_repo" not in sys.path:
    sys.path.insert(0, "/opt/trn_rl_repo")

import numpy as np

import concourse.bass as bass  # noqa: F401  (import establishes package init order)
import concourse.mybir as mybir
import concourse.tile as tile
from concourse import bacc, bass_utils

F32 = mybir.dt.float32
F32R = mybir.dt.float32r
BF16 = mybir.dt.bfloat16
AF = mybir.ActivationFunctionType

P = 128
N_CORES = 8
HEADS = 8
DH = 128
TOK = 4096           # b*n flattened token axis (attention mixes batches!)
SLICE = TOK // N_CORES   # 512 tokens per core
QD = 1024
CD = 768
INNER = 1024
KC = QD // P         # 8 qd chunks
CC = CD // P         # 6 cd chunks
TAU_SCALE = 1.5 * (DH ** -0.5)

_CACHE = {}


def _build():
    nc = bacc.Bacc(num_devices=N_CORES)

    xTs = nc.declare_dram_parameter("xTs", [QD, SLICE], F32R, isOutput=False)
    cTs = nc.declare_dram_parameter("cTs", [CD, SLICE], F32R, isOutput=False)
    Wq = nc.declare_dram_parameter("Wq", [QD, INNER], F32R, isOutput=False)
    Wk = nc.declare_dram_parameter("Wk", [CD, INNER], F32R, isOutput=False)
    Wv = nc.declare_dram_parameter("Wv", [CD, INNER], F32R, isOutput=False)
    Wout = nc.declare_dram_parameter("Wout", [INNER, QD], F32R, isOutput=False)
    boutT = nc.declare_dram_parameter("boutT", [P, KC], F32, isOutput=False)
    yT = nc.declare_dram_parameter("yT", [KC, P, SLICE], F32, isOutput=True)

    with tile.TileContext(nc) as tc:
        with (
            tc.tile_pool(name="const", bufs=1) as const,
            tc.tile_pool(name="sb", bufs=1) as sb,
            tc.tile_pool(name="ps", bufs=1, space="PSUM") as ps,
            tc.tile_pool(name="dram", bufs=1, space="DRAM") as dram,
        ):
            kv_in = dram.tile([2, HEADS, P, SLICE], BF16, name="kv_in")
            kv_g = dram.tile([N_CORES, 2, HEADS, P, SLICE], BF16,
                             addr_space="Shared", name="kv_g")

            ones_b = const.tile([P, 1], BF16, name="ones_b")
            nc.vector.memset(ones_b[:], 1.0)
            bout_sb = const.tile([P, KC], F32, name="bout_sb")
            nc.sync.dma_start(bout_sb[:], boutT[:, :])

            # ---- context slice (shared by K and V projections) ----
            cts = []
            for k in range(CC):
                t = sb.tile([P, SLICE], F32R, name=f"cts{k}", tag="cts", bufs=CC)
                nc.sync.dma_start(t[:], cTs[k * P:(k + 1) * P, :])
                cts.append(t)

            # ---- K projection: kT (head-major, [d, tok]) for this ctx slice ----
            for m in range(HEADS):
                wkm = sb.tile([P, CC, DH], F32R, name=f"wkm{m}", tag="wkm", bufs=2)
                nc.sync.dma_start(
                    wkm[:],
                    Wk.ap()[:, m * DH:(m + 1) * DH].rearrange(
                        "(k p) c -> p k c", p=P),
                )
                kps = ps.tile([P, SLICE], F32, name=f"kps{m}", tag="proj", bufs=2)
                for k in range(CC):
                    nc.tensor.matmul(kps[:], wkm[:, k],
                                     cts[k][:],
                                     start=(k == 0), stop=(k == CC - 1))
                ksb = sb.tile([P, SLICE], BF16, name=f"ksb{m}", tag="ksb", bufs=3)
                nc.vector.tensor_copy(ksb[:], kps[:])
                nc.sync.dma_start(kv_in[0, m], ksb[:])

            # ---- V projection: v (token-major, [tok, inner]) for this slice ----
            wvt = []
            for k in range(CC):
                t = sb.tile([P, INNER], F32R, name=f"wvt{k}", tag="wvt", bufs=CC)
                nc.sync.dma_start(t[:], Wv.ap()[k * P:(k + 1) * P, :])
                wvt.append(t)
            for tt in range(SLICE // P):  # 4 token tiles
                vsb = sb.tile([P, INNER], BF16, name=f"vsb{tt}", tag="vsb", bufs=2)
                for half in range(2):
                    vps = ps.tile([P, 512], F32, name=f"vps{tt}_{half}",
                                  tag="proj", bufs=2)
                    for k in range(CC):
                        nc.tensor.matmul(
                            vps[:],
                            cts[k][:, tt * P:(tt + 1) * P],
                            wvt[k][:, half * 512:(half + 1) * 512],
                            start=(k == 0), stop=(k == CC - 1))
                    nc.vector.tensor_copy(vsb[:, half * 512:(half + 1) * 512],
                                          vps[:])
                for h in range(HEADS):
                    nc.sync.dma_start(kv_in[1, h, :, tt * DH:(tt + 1) * DH],
                                      vsb[:, h * DH:(h + 1) * DH])

            # ---- AllGather K/V across the 8 cores ----
            nc.gpsimd.collective_compute(
                "AllGather", mybir.AluOpType.bypass,
                replica_groups=[list(range(N_CORES))],
                ins=[kv_in.opt()], outs=[kv_g.opt()],
            )

            # ---- Q projection (all heads, own token slice); overlaps AG ----
            xts = []
            for k in range(KC):
                t = sb.tile([P, SLICE], F32R, name=f"xts{k}", tag="xts", bufs=KC)
                nc.sync.dma_start(t[:], xTs[k * P:(k + 1) * P, :])
                xts.append(t)
            qsb = []
            for m in range(HEADS):
                wqm = sb.tile([P, KC, DH], F32R, name=f"wqm{m}", tag="wqm", bufs=2)
                nc.sync.dma_start(
                    wqm[:],
                    Wq.ap()[:, m * DH:(m + 1) * DH].rearrange(
                        "(k p) c -> p k c", p=P),
                )
                qps = ps.tile([P, SLICE], F32, name=f"qps{m}", tag="proj", bufs=2)
                for k in range(KC):
                    nc.tensor.matmul(qps[:], wqm[:, k],
                                     xts[k][:],
                                     start=(k == 0), stop=(k == KC - 1))
                qt = sb.tile([P, SLICE], BF16, name=f"qsb{m}", tag="qsb",
                             bufs=HEADS)
                nc.vector.tensor_copy(qt[:], qps[:])
                qsb.append(qt)

            # ---- attention, one head at a time over the full 4096 ctx ----
            JT = TOK // P          # 32 j-tiles
            GRP = 4                # j-tiles per exp group
            osb = []
            for h in range(HEADS):
                kh = sb.tile([P, TOK], BF16, name=f"kh{h}", tag="kh", bufs=2)
                vh = sb.tile([P, TOK], BF16, name=f"vh{h}", tag="vh", bufs=2)
                for r in range(N_CORES):
                    nc.sync.dma_start(kh[:, r * SLICE:(r + 1) * SLICE],
                                      kv_g[r, 0, h])
                    nc.sync.dma_start(vh[:, r * SLICE:(r + 1) * SLICE],
                                      kv_g[r, 1, h])
                pv_ps = ps.tile([P, SLICE], F32, name=f"pv{h}", tag="pv", bufs=1)
                rs_ps = ps.tile([1, SLICE], F32, name=f"rs{h}", tag="rs", bufs=1)
                for g in range(JT // GRP):
                    sim_ps = ps.tile([P, GRP * SLICE], F32, name=f"sim{h}_{g}",
                                     tag="sim", bufs=1)
                    for jj in range(GRP):
                        j = g * GRP + jj
                        nc.tensor.matmul(
                            sim_ps[:, jj * SLICE:(jj + 1) * SLICE],
                            kh[:, j * P:(j + 1) * P], qsb[h][:],
                            start=True, stop=True)
                    at = sb.tile([P, GRP * SLICE], BF16, name=f"at{h}_{g}",
                                 tag="at", bufs=3)
                    nc.scalar.activation(at[:], sim_ps[:], AF.Exp,
                                         scale=TAU_SCALE)
                    for jj in range(GRP):
                        j = g * GRP + jj
                        nc.tensor.matmul(pv_ps[:], vh[:, j * P:(j + 1) * P],
                                         at[:, jj * SLICE:(jj + 1) * SLICE],
                                         start=(j == 0), stop=(j == JT - 1))
                        nc.tensor.matmul(rs_ps[:], ones_b[:],
                                         at[:, jj * SLICE:(jj + 1) * SLICE],
                                         start=(j == 0), stop=(j == JT - 1))
                recip = sb.tile([1, SLICE], F32, name=f"recip{h}", tag="recip",
                                bufs=2)
                nc.vector.reciprocal(recip[:], rs_ps[:])
                bc = sb.tile([P, SLICE], F32, name=f"bc{h}", tag="bc", bufs=2)
                nc.gpsimd.partition_broadcast(bc[:], recip[:])
                ot = sb.tile([P, SLICE], F32R, name=f"osb{h}", tag="osb",
                             bufs=HEADS)
                nc.vector.tensor_tensor(ot[:], pv_ps[:], bc[:],
                                        mybir.AluOpType.mult)
                osb.append(ot)

            # ---- final projection: yT[cc] = Wout[:, cc]^T @ out^T + bout ----
            for cc in range(KC):
                wo = sb.tile([P, KC, DH], F32R, name=f"wo{cc}", tag="wo", bufs=2)
                nc.sync.dma_start(
                    wo[:],
                    Wout.ap()[:, cc * DH:(cc + 1) * DH].rearrange(
                        "(k p) c -> p k c", p=P),
                )
                yps = ps.tile([P, SLICE], F32, name=f"yps{cc}", tag="proj",
                              bufs=2)
                for ic in range(HEADS):
                    nc.tensor.matmul(yps[:], wo[:, ic],
                                     osb[ic][:],
                                     start=(ic == 0), stop=(ic == HEADS - 1))
                yt = sb.tile([P, SLICE], F32, name=f"yt{cc}", tag="yt", bufs=2)
                nc.scalar.activation(yt[:], yps[:], AF.Identity,
                                     bias=bout_sb[:, cc:cc + 1], scale=1.0)
                nc.sync.dma_start(yT.ap()[cc], yt[:])

    nc.compile()
    return nc


def _get_nc():
    if "nc" not in _CACHE:
        _CACHE["nc"] = _build()
    return _CACHE["nc"]


def _prep_in_maps(x, context, Wq, Wk, Wv, Wout, bout):
    x_f = np.ascontiguousarray(np.asarray(x, dtype=np.float32)).reshape(TOK, QD)
    c_f = np.ascontiguousarray(np.asarray(context, dtype=np.float32)).reshape(TOK, CD)
    Wq = np.ascontiguousarray(np.asarray(Wq, dtype=np.float32))
    Wk = np.ascontiguousarray(np.asarray(Wk, dtype=np.float32))
    Wv = np.ascontiguousarray(np.asarray(Wv, dtype=np.float32))
    Wout = np.ascontiguousarray(np.asarray(Wout, dtype=np.float32))
    boutT = np.ascontiguousarray(
        np.asarray(bout, dtype=np.float32).reshape(KC, P).T)
    in_maps = []
    for c in range(N_CORES):
        sl = slice(c * SLICE, (c + 1) * SLICE)
        in_maps.append({
            "xTs": np.ascontiguousarray(x_f[sl].T),
            "cTs": np.ascontiguousarray(c_f[sl].T),
            "Wq": Wq, "Wk": Wk, "Wv": Wv, "Wout": Wout, "boutT": boutT,
        })
    return in_maps


def _assemble(results):
    y = np.empty((TOK, QD), dtype=np.float32)
    for c in range(N_CORES):
        # yT: [KC, P, SLICE] -> y rows [c*SLICE, (c+1)*SLICE), cols cc*P+p
        yt = results[c]["yT"]
        y[c * SLICE:(c + 1) * SLICE] = (
            yt.transpose(2, 0, 1).reshape(SLICE, QD))
    return y.reshape(2, TOK // 2, QD)


def run(inputs, trace=False, **kw):
    nc = _get_nc()
    in_maps = _prep_in_maps(**inputs)
    res = bass_utils.run_bass_kernel_spmd(
        nc, in_maps, core_ids=list(range(N_CORES)), trace=trace, **kw)
    return _assemble(res.results), res


def kernel(**inputs):
    out, _ = run(inputs, trace=False)
    return out


# revision 3
# speedup vs baseline: 1.4393x; 1.4393x over previous
"""Trainium2 Bass kernel for nn_CrossAttention_31078383354530.

Reference computation (b=2, n=m=2048, qd=1024, cd=768, heads=8, dh=128):
    q = x @ Wq; k = ctx @ Wk; v = ctx @ Wv  (split into 8 heads of 128)
    sim = (q @ k^T) * dh**-0.5 over the FLATTENED (b*n)=4096 token axis
    attn = softmax((sim - mean)*1.5 + mean) == softmax(1.5*scale*(q@k^T))
        exactly (the mean-centering is a per-row constant shift)
    out = attn @ v -> merge heads -> y = out @ Wout + bout

Sharding (8 cores): context-token-sharded K/V projection + AllGather of the
bf16 K/V (all heads), then each core runs all 8 heads' attention for its own
512-query-token slice and its own final projection -> the output is a
disjoint row-slice per core (no reduction needed on host).

All matmul operands are bf16 (host converts the f32 inputs), accumulation is
f32 in PSUM. Softmax rowsum is an ones-stationary matmul (M=1) fused into
the PV stream; normalization (reciprocal + gpsimd partition_broadcast +
tensor_tensor multiply) runs off the critical path on SBUF copies so PSUM
banks recycle quickly and the PE stays dense (HAM stays at full clock).
"""

import sys

if "/opt/trn_rl_repo" not in sys.path:
    sys.path.insert(0, "/opt/trn_rl_repo")

import ml_dtypes
import numpy as np

import concourse.bass as bass  # noqa: F401
import concourse.mybir as mybir
import concourse.tile as tile
from concourse import bacc, bass_utils

F32 = mybir.dt.float32
BF16 = mybir.dt.bfloat16
AF = mybir.ActivationFunctionType

P = 128
N_CORES = 8
HEADS = 8
DH = 128
TOK = 4096             # b*n flattened token axis (attention mixes batches!)
SLICE = TOK // N_CORES  # 512 tokens per core
QD = 1024
CD = 768
INNER = 1024
KC = QD // P           # 8 qd chunks
CC = CD // P           # 6 cd chunks
JT = TOK // P          # 32 j-tiles per head
GRP = 3                # j-tiles per exp group ([128, 1536] psum, 3 banks)
TAU_SCALE = 1.5 * (DH ** -0.5)

_CACHE = {}


def _build():
    nc = bacc.Bacc(num_devices=N_CORES)

    xTs = nc.declare_dram_parameter("xTs", [QD, SLICE], BF16, isOutput=False)
    cTs = nc.declare_dram_parameter("cTs", [CD, SLICE], BF16, isOutput=False)
    Wq = nc.declare_dram_parameter("Wq", [QD, INNER], BF16, isOutput=False)
    Wk = nc.declare_dram_parameter("Wk", [CD, INNER], BF16, isOutput=False)
    Wv = nc.declare_dram_parameter("Wv", [CD, INNER], BF16, isOutput=False)
    Wout = nc.declare_dram_parameter("Wout", [INNER, QD], BF16, isOutput=False)
    boutT = nc.declare_dram_parameter("boutT", [P, KC], F32, isOutput=False)
    yT = nc.declare_dram_parameter("yT", [KC, P, SLICE], F32, isOutput=True)

    with tile.TileContext(nc) as tc:
        with (
            tc.tile_pool(name="const", bufs=1) as const,
            tc.tile_pool(name="sb", bufs=1) as sb,
            tc.tile_pool(name="ps", bufs=1, space="PSUM") as ps,
            tc.tile_pool(name="dram", bufs=1, space="DRAM") as dram,
        ):
            kv_in = dram.tile([2, HEADS, P, SLICE], BF16, name="kv_in")
            kv_g = dram.tile([N_CORES, 2, HEADS, P, SLICE], BF16,
                             addr_space="Shared", name="kv_g")

            ones_b = const.tile([P, 1], BF16, name="ones_b")
            nc.vector.memset(ones_b[:], 1.0)
            bout_sb = const.tile([P, KC], F32, name="bout_sb")
            nc.sync.dma_start(bout_sb[:], boutT[:, :])

            # ---- K/V projection inputs: contiguous row-chunk tiles ----
            cts = []
            for k in range(CC):
                t = sb.tile([P, SLICE], BF16, name=f"cts{k}", tag="cts", bufs=CC)
                nc.sync.dma_start(t[:], cTs[k * P:(k + 1) * P, :])
                cts.append(t)
            wkt = []
            for k in range(CC):
                t = sb.tile([P, INNER], BF16, name=f"wkt{k}", tag="wkt", bufs=CC)
                nc.sync.dma_start(t[:], Wk[k * P:(k + 1) * P, :])
                wkt.append(t)
            wvt = []
            for k in range(CC):
                t = sb.tile([P, INNER], BF16, name=f"wvt{k}", tag="wvt", bufs=CC)
                nc.sync.dma_start(t[:], Wv[k * P:(k + 1) * P, :])
                wvt.append(t)

            # ---- K projection: kT (head-major, [d, tok]) for this ctx slice
            # psum tag "sim" is shared with the attention phase (temporal reuse)
            for m in range(HEADS):
                kps = ps.tile([P, GRP * SLICE], F32, name=f"kps{m}", tag="sim",
                              bufs=2)
                for k in range(CC):
                    nc.tensor.matmul(kps[:, :SLICE], wkt[k][:, m * DH:(m + 1) * DH],
                                     cts[k][:],
                                     start=(k == 0), stop=(k == CC - 1))
                ksb = sb.tile([P, SLICE], BF16, name=f"ksb{m}", tag="ksb", bufs=3)
                nc.vector.tensor_copy(ksb[:], kps[:, :SLICE])
                nc.sync.dma_start(kv_in[0, m], ksb[:])

            # ---- V projection: v (token-major, [tok, inner]) for this slice
            for tt in range(SLICE // P):  # 4 token tiles
                vsb = sb.tile([P, INNER], BF16, name=f"vsb{tt}", tag="vsb", bufs=2)
                for half in range(2):
                    vps = ps.tile([P, GRP * SLICE], F32, name=f"vps{tt}_{half}",
                                  tag="sim", bufs=2)
                    for k in range(CC):
                        nc.tensor.matmul(
                            vps[:, :512],
                            cts[k][:, tt * P:(tt + 1) * P],
                            wvt[k][:, half * 512:(half + 1) * 512],
                            start=(k == 0), stop=(k == CC - 1))
                    nc.vector.tensor_copy(vsb[:, half * 512:(half + 1) * 512],
                                          vps[:, :512])
                for h in range(HEADS):
                    nc.sync.dma_start(kv_in[1, h, :, tt * DH:(tt + 1) * DH],
                                      vsb[:, h * DH:(h + 1) * DH])

            # ---- AllGather K/V across the 8 cores (overlaps q-projection) ----
            nc.gpsimd.collective_compute(
                "AllGather", mybir.AluOpType.bypass,
                replica_groups=[list(range(N_CORES))],
                ins=[kv_in.opt()], outs=[kv_g.opt()],
            )

            # ---- Q projection (all heads, own token slice); overlaps AG ----
            xts = []
            for k in range(KC):
                t = sb.tile([P, SLICE], BF16, name=f"xts{k}", tag="xts", bufs=KC)
                nc.sync.dma_start(t[:], xTs[k * P:(k + 1) * P, :])
                xts.append(t)
            wqt = []
            for k in range(KC):
                t = sb.tile([P, INNER], BF16, name=f"wqt{k}", tag="wqt", bufs=KC)
                nc.sync.dma_start(t[:], Wq[k * P:(k + 1) * P, :])
                wqt.append(t)
            qsb = []
            for m in range(HEADS):
                qps = ps.tile([P, GRP * SLICE], F32, name=f"qps{m}", tag="sim",
                              bufs=2)
                for k in range(KC):
                    nc.tensor.matmul(qps[:, :SLICE], wqt[k][:, m * DH:(m + 1) * DH],
                                     xts[k][:],
                                     start=(k == 0), stop=(k == KC - 1))
                qt = sb.tile([P, SLICE], BF16, name=f"qsb{m}", tag="qsb",
                             bufs=HEADS)
                nc.vector.tensor_copy(qt[:], qps[:, :SLICE])
                qsb.append(qt)

            # ---- attention, one head at a time over the full 4096 ctx ----
            groups = []
            j0 = 0
            while j0 < JT:
                groups.append(list(range(j0, min(j0 + GRP, JT))))
                j0 += GRP

            pvsb, rssb = [], []
            for h in range(HEADS):
                kh = sb.tile([P, TOK], BF16, name=f"kh{h}", tag="kh", bufs=2)
                vh = sb.tile([P, TOK], BF16, name=f"vh{h}", tag="vh", bufs=2)
                for r in range(N_CORES):
                    nc.sync.dma_start(kh[:, r * SLICE:(r + 1) * SLICE],
                                      kv_g[r, 0, h])
                    nc.sync.dma_start(vh[:, r * SLICE:(r + 1) * SLICE],
                                      kv_g[r, 1, h])
                pv_ps = ps.tile([P, SLICE], F32, name=f"pv{h}", tag="pv", bufs=1)
                rs_ps = ps.tile([1, SLICE], F32, name=f"rs{h}", tag="rs", bufs=1)
                for g, js in enumerate(groups):
                    sim_ps = ps.tile([P, GRP * SLICE], F32, name=f"sim{h}_{g}",
                                     tag="sim", bufs=2)
                    for jj, j in enumerate(js):
                        nc.tensor.matmul(
                            sim_ps[:, jj * SLICE:(jj + 1) * SLICE],
                            kh[:, j * P:(j + 1) * P], qsb[h][:],
                            start=True, stop=True)
                    at = sb.tile([P, GRP * SLICE], BF16, name=f"at{h}_{g}",
                                 tag="at", bufs=3)
                    nc.scalar.activation(at[:, :len(js) * SLICE],
                                         sim_ps[:, :len(js) * SLICE], AF.Exp,
                                         scale=TAU_SCALE)
                    for jj, j in enumerate(js):
                        nc.tensor.matmul(pv_ps[:], vh[:, j * P:(j + 1) * P],
                                         at[:, jj * SLICE:(jj + 1) * SLICE],
                                         start=(j == 0), stop=(j == JT - 1))
                        nc.tensor.matmul(rs_ps[:], ones_b[:],
                                         at[:, jj * SLICE:(jj + 1) * SLICE],
                                         start=(j == 0), stop=(j == JT - 1))
                # free the PSUM banks quickly; normalize later from SBUF
                pvc = sb.tile([P, SLICE], F32, name=f"pvsb{h}", tag="pvsb",
                              bufs=HEADS)
                nc.vector.tensor_copy(pvc[:], pv_ps[:])
                pvsb.append(pvc)
                rsc = sb.tile([1, SLICE], F32, name=f"rssb{h}", tag="rssb",
                              bufs=HEADS)
                nc.vector.tensor_copy(rsc[:], rs_ps[:])
                rssb.append(rsc)

            # ---- normalization (off critical path) ----
            osb = []
            for h in range(HEADS):
                recip = sb.tile([1, SLICE], F32, name=f"recip{h}", tag="recip",
                                bufs=2)
                nc.vector.reciprocal(recip[:], rssb[h][:])
                bc = sb.tile([P, SLICE], F32, name=f"bc{h}", tag="bc", bufs=2)
                nc.gpsimd.partition_broadcast(bc[:], recip[:])
                ot = sb.tile([P, SLICE], BF16, name=f"osb{h}", tag="osb",
                             bufs=HEADS)
                nc.vector.tensor_tensor(ot[:], pvsb[h][:], bc[:],
                                        mybir.AluOpType.mult)
                osb.append(ot)

            # ---- final projection: yT[cc] = Wout[:, cc]^T @ out^T + bout ----
            for cc in range(KC):
                wo = sb.tile([P, KC, DH], BF16, name=f"wo{cc}", tag="wo", bufs=2)
                nc.sync.dma_start(
                    wo[:],
                    Wout.ap()[:, cc * DH:(cc + 1) * DH].rearrange(
                        "(k p) c -> p k c", p=P),
                )
                yps = ps.tile([P, SLICE], F32, name=f"yps{cc}", tag="pv", bufs=1)
                for ic in range(HEADS):
                    nc.tensor.matmul(yps[:], wo[:, ic], osb[ic][:],
                                     start=(ic == 0), stop=(ic == HEADS - 1))
                yt = sb.tile([P, SLICE], F32, name=f"yt{cc}", tag="yt", bufs=2)
                nc.scalar.activation(yt[:], yps[:], AF.Identity,
                                     bias=bout_sb[:, cc:cc + 1], scale=1.0)
                nc.sync.dma_start(yT.ap()[cc], yt[:])

    nc.compile()
    return nc


def _get_nc():
    if "nc" not in _CACHE:
        _CACHE["nc"] = _build()
    return _CACHE["nc"]


def _bf16(a):
    return np.ascontiguousarray(np.asarray(a, np.float32).astype(ml_dtypes.bfloat16))


def _prep_in_maps(x, context, Wq, Wk, Wv, Wout, bout):
    x_f = np.asarray(x, dtype=np.float32).reshape(TOK, QD)
    c_f = np.asarray(context, dtype=np.float32).reshape(TOK, CD)
    Wq = _bf16(Wq)
    Wk = _bf16(Wk)
    Wv = _bf16(Wv)
    Wout = _bf16(Wout)
    boutT = np.ascontiguousarray(
        np.asarray(bout, dtype=np.float32).reshape(KC, P).T)
    in_maps = []
    for c in range(N_CORES):
        sl = slice(c * SLICE, (c + 1) * SLICE)
        in_maps.append({
            "xTs": _bf16(x_f[sl].T),
            "cTs": _bf16(c_f[sl].T),
            "Wq": Wq, "Wk": Wk, "Wv": Wv, "Wout": Wout, "boutT": boutT,
        })
    return in_maps


def _assemble(results):
    y = np.empty((TOK, QD), dtype=np.float32)
    for c in range(N_CORES):
        yt = results[c]["yT"]   # [KC, P, SLICE]
        y[c * SLICE:(c + 1) * SLICE] = (
            yt.transpose(2, 0, 1).reshape(SLICE, QD))
    return y.reshape(2, TOK // 2, QD)


def run(inputs, trace=False, **kw):
    nc = _get_nc()
    in_maps = _prep_in_maps(**inputs)
    res = bass_utils.run_bass_kernel_spmd(
        nc, in_maps, core_ids=list(range(N_CORES)), trace=trace, **kw)
    return _assemble(res.results), res


def kernel(**inputs):
    out, _ = run(inputs, trace=False)
    return out


# revision 8
# speedup vs baseline: 1.5753x; 1.0945x over previous
"""Trainium2 Bass kernel for nn_CrossAttention_31078383354530.

Reference computation (b=2, n=m=2048, qd=1024, cd=768, heads=8, dh=128):
    q = x @ Wq; k = ctx @ Wk; v = ctx @ Wv  (split into 8 heads of 128)
    sim = (q @ k^T) * dh**-0.5 over the FLATTENED (b*n)=4096 token axis
    attn = softmax((sim - mean)*1.5 + mean) == softmax(1.5*scale*(q@k^T))
        exactly (the mean-centering is a per-row constant shift)
    out = attn @ v -> merge heads -> y = out @ Wout + bout

Sharding (8 cores): context-token-sharded K/V projection + AllGather of the
bf16 K/V (all heads), then each core runs all 8 heads' attention for its own
512-query-token slice and its own final projection -> the output is a
disjoint row-slice per core (no reduction needed on host).

All matmul operands are bf16 (host converts the f32 inputs), accumulation is
f32 in PSUM. Softmax rowsum is an ones-stationary matmul (M=1) fused into
the PV stream; normalization (reciprocal + gpsimd partition_broadcast +
tensor_tensor multiply) runs off the critical path on SBUF copies so PSUM
banks recycle quickly and the PE stays dense (HAM stays at full clock).
"""

import sys

if "/opt/trn_rl_repo" not in sys.path:
    sys.path.insert(0, "/opt/trn_rl_repo")

import ml_dtypes
import numpy as np

import concourse.bass as bass  # noqa: F401
import concourse.mybir as mybir
import concourse.tile as tile
from concourse import bacc, bass_utils

F32 = mybir.dt.float32
BF16 = mybir.dt.bfloat16
AF = mybir.ActivationFunctionType

P = 128
N_CORES = 8
HEADS = 8
DH = 128
TOK = 4096             # b*n flattened token axis (attention mixes batches!)
SLICE = TOK // N_CORES  # 512 tokens per core
QD = 1024
CD = 768
INNER = 1024
KC = QD // P           # 8 qd chunks
CC = CD // P           # 6 cd chunks
JT = TOK // P          # 32 j-tiles per head
GRP = 3                # j-tiles per exp group ([128, 1536] psum, 3 banks)
TAU_SCALE = 1.5 * (DH ** -0.5)

_CACHE = {}


def _build():
    nc = bacc.Bacc(num_devices=N_CORES)

    xTs = nc.declare_dram_parameter("xTs", [QD, SLICE], BF16, isOutput=False)
    cTs = nc.declare_dram_parameter("cTs", [CD, SLICE], BF16, isOutput=False)
    Wq = nc.declare_dram_parameter("Wq", [QD, INNER], BF16, isOutput=False)
    Wk = nc.declare_dram_parameter("Wk", [CD, INNER], BF16, isOutput=False)
    Wv = nc.declare_dram_parameter("Wv", [CD, INNER], BF16, isOutput=False)
    Wout = nc.declare_dram_parameter("Wout", [INNER, QD], BF16, isOutput=False)
    boutT = nc.declare_dram_parameter("boutT", [P, KC], F32, isOutput=False)
    yT = nc.declare_dram_parameter("yT", [KC, P, SLICE], F32, isOutput=True)

    with tile.TileContext(nc) as tc:
        with (
            tc.tile_pool(name="const", bufs=1) as const,
            tc.tile_pool(name="sb", bufs=1) as sb,
            tc.tile_pool(name="ps", bufs=1, space="PSUM") as ps,
            tc.tile_pool(name="dram", bufs=1, space="DRAM") as dram,
        ):
            # per-head-pair bounce buffers -> 4 pipelined AllGathers, so the
            # attention on early heads overlaps the later transfers
            NP = HEADS // 2
            kv_in = [dram.tile([2, 2, P, SLICE], BF16, name=f"kv_in{p}")
                     for p in range(NP)]
            kv_g = [dram.tile([N_CORES, 2, 2, P, SLICE], BF16,
                              addr_space="Shared", name=f"kv_g{p}")
                    for p in range(NP)]

            ones_b = const.tile([P, 1], BF16, name="ones_b")
            nc.vector.memset(ones_b[:], 1.0)
            bout_sb = const.tile([P, KC], F32, name="bout_sb")
            nc.sync.dma_start(bout_sb[:], boutT[:, :])

            # ---- K/V projection inputs: contiguous row-chunk tiles ----
            cts = []
            for k in range(CC):
                t = sb.tile([P, SLICE], BF16, name=f"cts{k}", tag="cts", bufs=CC)
                nc.sync.dma_start(t[:], cTs[k * P:(k + 1) * P, :])
                cts.append(t)
            wkt = []
            for k in range(CC):
                t = sb.tile([P, INNER], BF16, name=f"wkt{k}", tag="wkt", bufs=CC)
                nc.sync.dma_start(t[:], Wk[k * P:(k + 1) * P, :])
                wkt.append(t)
            wvt = []
            for k in range(CC):
                t = sb.tile([P, INNER], BF16, name=f"wvt{k}", tag="wvt", bufs=CC)
                nc.sync.dma_start(t[:], Wv[k * P:(k + 1) * P, :])
                wvt.append(t)

            # ---- K projection: kT (head-major, [d, tok]) for this ctx slice
            # psum tag "sim" is shared with the attention phase (temporal reuse)
            for m in range(HEADS):
                kps = ps.tile([P, GRP * SLICE], F32, name=f"kps{m}", tag="sim",
                              bufs=2)
                for k in range(CC):
                    nc.tensor.matmul(kps[:, :SLICE], wkt[k][:, m * DH:(m + 1) * DH],
                                     cts[k][:],
                                     start=(k == 0), stop=(k == CC - 1))
                ksb = sb.tile([P, SLICE], BF16, name=f"ksb{m}", tag="ksb", bufs=3)
                nc.vector.tensor_copy(ksb[:], kps[:, :SLICE])
                nc.sync.dma_start(kv_in[m // 2][0, m % 2], ksb[:])

            # ---- V projection: v (token-major, [tok, inner]) for this slice
            # half-major so heads 0..3 (pairs 0,1) are ready before heads 4..7
            vsb = [sb.tile([P, INNER], BF16, name=f"vsb{tt}", tag="vsb", bufs=4)
                   for tt in range(SLICE // P)]
            for half in range(2):
                for tt in range(SLICE // P):  # 4 token tiles
                    vps = ps.tile([P, GRP * SLICE], F32, name=f"vps{tt}_{half}",
                                  tag="sim", bufs=2)
                    for k in range(CC):
                        nc.tensor.matmul(
                            vps[:, :512],
                            cts[k][:, tt * P:(tt + 1) * P],
                            wvt[k][:, half * 512:(half + 1) * 512],
                            start=(k == 0), stop=(k == CC - 1))
                    nc.vector.tensor_copy(vsb[tt][:, half * 512:(half + 1) * 512],
                                          vps[:, :512])
                for h in range(4 * half, 4 * half + 4):
                    for tt in range(SLICE // P):
                        nc.sync.dma_start(
                            kv_in[h // 2][1, h % 2, :, tt * DH:(tt + 1) * DH],
                            vsb[tt][:, h * DH:(h + 1) * DH])
                # ---- pipelined AllGathers (2 heads per collective) ----
                for p in (2 * half, 2 * half + 1):
                    nc.gpsimd.collective_compute(
                        "AllGather", mybir.AluOpType.bypass,
                        replica_groups=[list(range(N_CORES))],
                        ins=[kv_in[p].opt()], outs=[kv_g[p].opt()],
                    )

            # ---- Q projection (all heads, own token slice); overlaps AG ----
            xts = []
            for k in range(KC):
                t = sb.tile([P, SLICE], BF16, name=f"xts{k}", tag="xts", bufs=KC)
                nc.sync.dma_start(t[:], xTs[k * P:(k + 1) * P, :])
                xts.append(t)
            wqt = []
            for k in range(KC):
                t = sb.tile([P, INNER], BF16, name=f"wqt{k}", tag="wqt", bufs=KC)
                nc.sync.dma_start(t[:], Wq[k * P:(k + 1) * P, :])
                wqt.append(t)
            qsb = []
            for m in range(HEADS):
                qps = ps.tile([P, GRP * SLICE], F32, name=f"qps{m}", tag="sim",
                              bufs=2)
                for k in range(KC):
                    nc.tensor.matmul(qps[:, :SLICE], wqt[k][:, m * DH:(m + 1) * DH],
                                     xts[k][:],
                                     start=(k == 0), stop=(k == KC - 1))
                qt = sb.tile([P, SLICE], BF16, name=f"qsb{m}", tag="qsb",
                             bufs=HEADS)
                nc.vector.tensor_copy(qt[:], qps[:, :SLICE])
                qsb.append(qt)

            # ---- attention, one head at a time over the full 4096 ctx ----
            groups = []
            j0 = 0
            while j0 < JT:
                groups.append(list(range(j0, min(j0 + GRP, JT))))
                j0 += GRP

            osb = [None] * HEADS
            for h in range(HEADS):
                kh = sb.tile([P, TOK], BF16, name=f"kh{h}", tag="kh", bufs=2)
                vh = sb.tile([P, TOK], BF16, name=f"vh{h}", tag="vh", bufs=2)
                for r in range(N_CORES):
                    nc.sync.dma_start(kh[:, r * SLICE:(r + 1) * SLICE],
                                      kv_g[h // 2][r, 0, h % 2])
                    nc.sync.dma_start(vh[:, r * SLICE:(r + 1) * SLICE],
                                      kv_g[h // 2][r, 1, h % 2])
                pv_ps = ps.tile([P, SLICE], F32, name=f"pv{h}", tag="pv", bufs=1)
                rs_ps = ps.tile([1, SLICE], F32, name=f"rs{h}", tag="rs", bufs=1)
                for g, js in enumerate(groups):
                    sim_ps = ps.tile([P, GRP * SLICE], F32, name=f"sim{h}_{g}",
                                     tag="sim", bufs=2)
                    for jj, j in enumerate(js):
                        nc.tensor.matmul(
                            sim_ps[:, jj * SLICE:(jj + 1) * SLICE],
                            kh[:, j * P:(j + 1) * P], qsb[h][:],
                            start=True, stop=True)
                    at = sb.tile([P, GRP * SLICE], BF16, name=f"at{h}_{g}",
                                 tag="at", bufs=3)
                    nc.scalar.activation(at[:, :len(js) * SLICE],
                                         sim_ps[:, :len(js) * SLICE], AF.Exp,
                                         scale=TAU_SCALE)
                    for jj, j in enumerate(js):
                        nc.tensor.matmul(pv_ps[:], vh[:, j * P:(j + 1) * P],
                                         at[:, jj * SLICE:(jj + 1) * SLICE],
                                         start=(j == 0), stop=(j == JT - 1))
                        nc.tensor.matmul(rs_ps[:], ones_b[:],
                                         at[:, jj * SLICE:(jj + 1) * SLICE],
                                         start=(j == 0), stop=(j == JT - 1))
                # free the PSUM banks quickly; normalization runs off the
                # critical path on SBUF copies (overlaps the next head)
                pvc = sb.tile([P, SLICE], F32, name=f"pvsb{h}", tag="pvsb",
                              bufs=2)
                nc.vector.tensor_copy(pvc[:], pv_ps[:])
                rsc = sb.tile([1, SLICE], F32, name=f"rssb{h}", tag="rssb",
                              bufs=2)
                nc.vector.tensor_copy(rsc[:], rs_ps[:])
                recip = sb.tile([1, SLICE], F32, name=f"recip{h}", tag="recip",
                                bufs=2)
                nc.vector.reciprocal(recip[:], rsc[:])
                bc = sb.tile([P, SLICE], F32, name=f"bc{h}", tag="bc", bufs=2)
                nc.gpsimd.partition_broadcast(bc[:], recip[:])
                ot = sb.tile([P, SLICE], BF16, name=f"osb{h}", tag="osb",
                             bufs=HEADS)
                nc.vector.tensor_tensor(ot[:], pvc[:], bc[:],
                                        mybir.AluOpType.mult)
                osb[h] = ot

            # ---- final projection: yT[cc] = Wout[:, cc]^T @ out^T + bout ----
            for cc in range(KC):
                wo = sb.tile([P, KC, DH], BF16, name=f"wo{cc}", tag="wo", bufs=4)
                nc.sync.dma_start(
                    wo[:],
                    Wout.ap()[:, cc * DH:(cc + 1) * DH].rearrange(
                        "(k p) c -> p k c", p=P),
                )
                # alternate between the two single-bank psum tags so
                # consecutive output chunks double-buffer
                yps = ps.tile([P, SLICE], F32, name=f"yps{cc}",
                              tag=("pv" if cc % 2 == 0 else "rs"), bufs=1)
                for ic in range(HEADS):
                    nc.tensor.matmul(yps[:], wo[:, ic], osb[ic][:],
                                     start=(ic == 0), stop=(ic == HEADS - 1))
                yt = sb.tile([P, SLICE], F32, name=f"yt{cc}", tag="yt", bufs=2)
                nc.scalar.activation(yt[:], yps[:], AF.Identity,
                                     bias=bout_sb[:, cc:cc + 1], scale=1.0)
                nc.sync.dma_start(yT.ap()[cc], yt[:])

    nc.compile()
    return nc


def _get_nc():
    if "nc" not in _CACHE:
        _CACHE["nc"] = _build()
    return _CACHE["nc"]


def _bf16(a):
    return np.ascontiguousarray(np.asarray(a, np.float32).astype(ml_dtypes.bfloat16))


def _prep_in_maps(x, context, Wq, Wk, Wv, Wout, bout):
    x_f = np.asarray(x, dtype=np.float32).reshape(TOK, QD)
    c_f = np.asarray(context, dtype=np.float32).reshape(TOK, CD)
    Wq = _bf16(Wq)
    Wk = _bf16(Wk)
    Wv = _bf16(Wv)
    Wout = _bf16(Wout)
    boutT = np.ascontiguousarray(
        np.asarray(bout, dtype=np.float32).reshape(KC, P).T)
    in_maps = []
    for c in range(N_CORES):
        sl = slice(c * SLICE, (c + 1) * SLICE)
        in_maps.append({
            "xTs": _bf16(x_f[sl].T),
            "cTs": _bf16(c_f[sl].T),
            "Wq": Wq, "Wk": Wk, "Wv": Wv, "Wout": Wout, "boutT": boutT,
        })
    return in_maps


def _assemble(results):
    y = np.empty((TOK, QD), dtype=np.float32)
    for c in range(N_CORES):
        yt = results[c]["yT"]   # [KC, P, SLICE]
        y[c * SLICE:(c + 1) * SLICE] = (
            yt.transpose(2, 0, 1).reshape(SLICE, QD))
    return y.reshape(2, TOK // 2, QD)


def run(inputs, trace=False, **kw):
    nc = _get_nc()
    in_maps = _prep_in_maps(**inputs)
    res = bass_utils.run_bass_kernel_spmd(
        nc, in_maps, core_ids=list(range(N_CORES)), trace=trace, **kw)
    return _assemble(res.results), res


def kernel(**inputs):
    out, _ = run(inputs, trace=False)
    return out


# revision 10
# speedup vs baseline: 1.6008x; 1.0162x over previous
"""Trainium2 Bass kernel for nn_CrossAttention_31078383354530.

Reference computation (b=2, n=m=2048, qd=1024, cd=768, heads=8, dh=128):
    q = x @ Wq; k = ctx @ Wk; v = ctx @ Wv  (split into 8 heads of 128)
    sim = (q @ k^T) * dh**-0.5 over the FLATTENED (b*n)=4096 token axis
    attn = softmax((sim - mean)*1.5 + mean) == softmax(1.5*scale*(q@k^T))
        exactly (the mean-centering is a per-row constant shift)
    out = attn @ v -> merge heads -> y = out @ Wout + bout

Sharding (8 cores): context-token-sharded K/V projection + AllGather of the
bf16 K/V (all heads), then each core runs all 8 heads' attention for its own
512-query-token slice and its own final projection -> the output is a
disjoint row-slice per core (no reduction needed on host).

All matmul operands are bf16 (host converts the f32 inputs), accumulation is
f32 in PSUM. Softmax rowsum is an ones-stationary matmul (M=1) fused into
the PV stream; normalization (reciprocal + gpsimd partition_broadcast +
tensor_tensor multiply) runs off the critical path on SBUF copies so PSUM
banks recycle quickly and the PE stays dense (HAM stays at full clock).
"""

import sys

if "/opt/trn_rl_repo" not in sys.path:
    sys.path.insert(0, "/opt/trn_rl_repo")

import ml_dtypes
import numpy as np

import concourse.bass as bass  # noqa: F401
import concourse.mybir as mybir
import concourse.tile as tile
from concourse import bacc, bass_utils

F32 = mybir.dt.float32
BF16 = mybir.dt.bfloat16
AF = mybir.ActivationFunctionType

P = 128
N_CORES = 8
HEADS = 8
DH = 128
TOK = 4096             # b*n flattened token axis (attention mixes batches!)
SLICE = TOK // N_CORES  # 512 tokens per core
QD = 1024
CD = 768
INNER = 1024
KC = QD // P           # 8 qd chunks
CC = CD // P           # 6 cd chunks
JT = TOK // P          # 32 j-tiles per head
GRP = 3                # j-tiles per exp group ([128, 1536] psum, 3 banks)
TAU_SCALE = 1.5 * (DH ** -0.5)

_CACHE = {}


def _build():
    nc = bacc.Bacc(num_devices=N_CORES)

    xTs = nc.declare_dram_parameter("xTs", [QD, SLICE], BF16, isOutput=False)
    cTs = nc.declare_dram_parameter("cTs", [CD, SLICE], BF16, isOutput=False)
    Wq = nc.declare_dram_parameter("Wq", [QD, INNER], BF16, isOutput=False)
    Wk = nc.declare_dram_parameter("Wk", [CD, INNER], BF16, isOutput=False)
    Wv = nc.declare_dram_parameter("Wv", [CD, INNER], BF16, isOutput=False)
    Wout = nc.declare_dram_parameter("Wout", [INNER, QD], BF16, isOutput=False)
    boutT = nc.declare_dram_parameter("boutT", [P, KC], F32, isOutput=False)
    yT = nc.declare_dram_parameter("yT", [KC, P, SLICE], F32, isOutput=True)

    with tile.TileContext(nc) as tc:
        with (
            tc.tile_pool(name="const", bufs=1) as const,
            tc.tile_pool(name="sb", bufs=1) as sb,
            tc.tile_pool(name="ps", bufs=1, space="PSUM") as ps,
            tc.tile_pool(name="dram", bufs=1, space="DRAM") as dram,
        ):
            # per-head-pair bounce buffers -> 4 pipelined AllGathers, so the
            # attention on early heads overlaps the later transfers
            NP = HEADS // 2
            kv_in = [dram.tile([2, 2, P, SLICE], BF16, name=f"kv_in{p}")
                     for p in range(NP)]
            kv_g = [dram.tile([N_CORES, 2, 2, P, SLICE], BF16,
                              addr_space="Shared", name=f"kv_g{p}")
                    for p in range(NP)]

            ones_b = const.tile([P, 1], BF16, name="ones_b")
            nc.vector.memset(ones_b[:], 1.0)
            bout_sb = const.tile([P, KC], F32, name="bout_sb")
            nc.sync.dma_start(bout_sb[:], boutT[:, :])

            # ---- K/V projection inputs: contiguous row-chunk tiles ----
            cts = []
            for k in range(CC):
                t = sb.tile([P, SLICE], BF16, name=f"cts{k}", tag="cts", bufs=CC)
                nc.sync.dma_start(t[:], cTs[k * P:(k + 1) * P, :])
                cts.append(t)
            wkt = []
            for k in range(CC):
                t = sb.tile([P, INNER], BF16, name=f"wkt{k}", tag="wkt", bufs=CC)
                nc.sync.dma_start(t[:], Wk[k * P:(k + 1) * P, :])
                wkt.append(t)
            wvt = []
            for k in range(CC):
                t = sb.tile([P, INNER], BF16, name=f"wvt{k}", tag="wvt", bufs=CC)
                nc.sync.dma_start(t[:], Wv[k * P:(k + 1) * P, :])
                wvt.append(t)

            # ---- K projection: kT (head-major, [d, tok]) for this ctx slice
            # psum tag "sim" is shared with the attention phase (temporal reuse)
            for m in range(HEADS):
                kps = ps.tile([P, GRP * SLICE], F32, name=f"kps{m}", tag="sim",
                              bufs=2)
                for k in range(CC):
                    nc.tensor.matmul(kps[:, :SLICE], wkt[k][:, m * DH:(m + 1) * DH],
                                     cts[k][:],
                                     start=(k == 0), stop=(k == CC - 1))
                ksb = sb.tile([P, SLICE], BF16, name=f"ksb{m}", tag="ksb", bufs=3)
                nc.vector.tensor_copy(ksb[:], kps[:, :SLICE])
                nc.sync.dma_start(kv_in[m // 2][0, m % 2], ksb[:])

            # ---- V projection: v (token-major, [tok, inner]) for this slice
            # half-major so heads 0..3 (pairs 0,1) are ready before heads 4..7
            vsb = [sb.tile([P, INNER], BF16, name=f"vsb{tt}", tag="vsb", bufs=4)
                   for tt in range(SLICE // P)]
            for half in range(2):
                for tt in range(SLICE // P):  # 4 token tiles
                    vps = ps.tile([P, GRP * SLICE], F32, name=f"vps{tt}_{half}",
                                  tag="sim", bufs=2)
                    for k in range(CC):
                        nc.tensor.matmul(
                            vps[:, :512],
                            cts[k][:, tt * P:(tt + 1) * P],
                            wvt[k][:, half * 512:(half + 1) * 512],
                            start=(k == 0), stop=(k == CC - 1))
                    nc.vector.tensor_copy(vsb[tt][:, half * 512:(half + 1) * 512],
                                          vps[:, :512])
                for h in range(4 * half, 4 * half + 4):
                    for tt in range(SLICE // P):
                        nc.sync.dma_start(
                            kv_in[h // 2][1, h % 2, :, tt * DH:(tt + 1) * DH],
                            vsb[tt][:, h * DH:(h + 1) * DH])
                # ---- pipelined AllGathers (2 heads per collective) ----
                for p in (2 * half, 2 * half + 1):
                    nc.gpsimd.collective_compute(
                        "AllGather", mybir.AluOpType.bypass,
                        replica_groups=[list(range(N_CORES))],
                        ins=[kv_in[p].opt()], outs=[kv_g[p].opt()],
                    )

            # ---- Q projection (all heads, own token slice); overlaps AG ----
            xts = []
            for k in range(KC):
                t = sb.tile([P, SLICE], BF16, name=f"xts{k}", tag="xts", bufs=KC)
                nc.sync.dma_start(t[:], xTs[k * P:(k + 1) * P, :])
                xts.append(t)
            wqt = []
            for k in range(KC):
                t = sb.tile([P, INNER], BF16, name=f"wqt{k}", tag="wqt", bufs=KC)
                nc.sync.dma_start(t[:], Wq[k * P:(k + 1) * P, :])
                wqt.append(t)
            qsb = []
            for m in range(HEADS):
                qps = ps.tile([P, GRP * SLICE], F32, name=f"qps{m}", tag="sim",
                              bufs=2)
                for k in range(KC):
                    nc.tensor.matmul(qps[:, :SLICE], wqt[k][:, m * DH:(m + 1) * DH],
                                     xts[k][:],
                                     start=(k == 0), stop=(k == KC - 1))
                qt = sb.tile([P, SLICE], BF16, name=f"qsb{m}", tag="qsb",
                             bufs=HEADS)
                nc.vector.tensor_copy(qt[:], qps[:, :SLICE])
                qsb.append(qt)

            # ---- attention, one head at a time over the full 4096 ctx ----
            groups = []
            j0 = 0
            while j0 < JT:
                groups.append(list(range(j0, min(j0 + GRP, JT))))
                j0 += GRP

            osb = [None] * HEADS
            for h in range(HEADS):
                kh = sb.tile([P, TOK], BF16, name=f"kh{h}", tag="kh", bufs=2)
                vh = sb.tile([P, TOK], BF16, name=f"vh{h}", tag="vh", bufs=2)
                for r in range(N_CORES):
                    nc.sync.dma_start(kh[:, r * SLICE:(r + 1) * SLICE],
                                      kv_g[h // 2][r, 0, h % 2])
                    nc.sync.dma_start(vh[:, r * SLICE:(r + 1) * SLICE],
                                      kv_g[h // 2][r, 1, h % 2])
                pv_ps = ps.tile([P, SLICE], F32, name=f"pv{h}", tag="pv", bufs=1)
                rs_ps = ps.tile([1, SLICE], F32, name=f"rs{h}", tag="rs", bufs=1)
                for g, js in enumerate(groups):
                    sim_ps = ps.tile([P, GRP * SLICE], F32, name=f"sim{h}_{g}",
                                     tag="sim", bufs=2)
                    for jj, j in enumerate(js):
                        nc.tensor.matmul(
                            sim_ps[:, jj * SLICE:(jj + 1) * SLICE],
                            kh[:, j * P:(j + 1) * P], qsb[h][:],
                            start=True, stop=True)
                    at = sb.tile([P, GRP * SLICE], BF16, name=f"at{h}_{g}",
                                 tag="at", bufs=3)
                    nc.scalar.activation(at[:, :len(js) * SLICE],
                                         sim_ps[:, :len(js) * SLICE], AF.Exp,
                                         scale=TAU_SCALE)
                    for jj, j in enumerate(js):
                        nc.tensor.matmul(pv_ps[:], vh[:, j * P:(j + 1) * P],
                                         at[:, jj * SLICE:(jj + 1) * SLICE],
                                         start=(j == 0), stop=(j == JT - 1))
                        nc.tensor.matmul(rs_ps[:], ones_b[:],
                                         at[:, jj * SLICE:(jj + 1) * SLICE],
                                         start=(j == 0), stop=(j == JT - 1))
                # free the PSUM banks quickly; normalization runs off the
                # critical path on SBUF copies (overlaps the next head)
                pvc = sb.tile([P, SLICE], F32, name=f"pvsb{h}", tag="pvsb",
                              bufs=2)
                nc.vector.tensor_copy(pvc[:], pv_ps[:])
                rsc = sb.tile([1, SLICE], F32, name=f"rssb{h}", tag="rssb",
                              bufs=2)
                nc.vector.tensor_copy(rsc[:], rs_ps[:])
                recip = sb.tile([1, SLICE], F32, name=f"recip{h}", tag="recip",
                                bufs=2)
                nc.vector.reciprocal(recip[:], rsc[:])
                bc = sb.tile([P, SLICE], F32, name=f"bc{h}", tag="bc", bufs=2)
                nc.gpsimd.partition_broadcast(bc[:], recip[:])
                ot = sb.tile([P, SLICE], BF16, name=f"osb{h}", tag="osb",
                             bufs=HEADS)
                nc.vector.tensor_tensor(ot[:], pvc[:], bc[:],
                                        mybir.AluOpType.mult)
                osb[h] = ot

            # ---- final projection: yT[cc] = Wout[:, cc]^T @ out^T + bout ----
            for cc in range(KC):
                wo = sb.tile([P, KC, DH], BF16, name=f"wo{cc}", tag="wo", bufs=4)
                nc.sync.dma_start(
                    wo[:],
                    Wout.ap()[:, cc * DH:(cc + 1) * DH].rearrange(
                        "(k p) c -> p k c", p=P),
                )
                # alternate between the two single-bank psum tags so
                # consecutive output chunks double-buffer
                yps = ps.tile([P, SLICE], F32, name=f"yps{cc}",
                              tag=("pv" if cc % 2 == 0 else "rs"), bufs=1)
                for ic in range(HEADS):
                    nc.tensor.matmul(yps[:], wo[:, ic], osb[ic][:],
                                     start=(ic == 0), stop=(ic == HEADS - 1))
                yt = sb.tile([P, SLICE], F32, name=f"yt{cc}", tag="yt", bufs=2)
                nc.scalar.activation(yt[:], yps[:], AF.Identity,
                                     bias=bout_sb[:, cc:cc + 1], scale=1.0)
                nc.sync.dma_start(yT.ap()[cc], yt[:])

    nc.compile()
    return nc


def _get_nc():
    if "nc" not in _CACHE:
        _CACHE["nc"] = _build()
    return _CACHE["nc"]


def _bf16(a):
    return np.ascontiguousarray(np.asarray(a, np.float32).astype(ml_dtypes.bfloat16))


def _prep_in_maps(x, context, Wq, Wk, Wv, Wout, bout):
    x_f = np.asarray(x, dtype=np.float32).reshape(TOK, QD)
    c_f = np.asarray(context, dtype=np.float32).reshape(TOK, CD)
    Wq = _bf16(Wq)
    Wk = _bf16(Wk)
    Wv = _bf16(Wv)
    Wout = _bf16(Wout)
    boutT = np.ascontiguousarray(
        np.asarray(bout, dtype=np.float32).reshape(KC, P).T)
    in_maps = []
    for c in range(N_CORES):
        sl = slice(c * SLICE, (c + 1) * SLICE)
        in_maps.append({
            "xTs": _bf16(x_f[sl].T),
            "cTs": _bf16(c_f[sl].T),
            "Wq": Wq, "Wk": Wk, "Wv": Wv, "Wout": Wout, "boutT": boutT,
        })
    return in_maps


def _assemble(results):
    y = np.empty((TOK, QD), dtype=np.float32)
    for c in range(N_CORES):
        yt = results[c]["yT"]   # [KC, P, SLICE]
        y[c * SLICE:(c + 1) * SLICE] = (
            yt.transpose(2, 0, 1).reshape(SLICE, QD))
    return y.reshape(2, TOK // 2, QD)


def run(inputs, trace=False, **kw):
    nc = _get_nc()
    in_maps = _prep_in_maps(**inputs)
    res = bass_utils.run_bass_kernel_spmd(
        nc, in_maps, core_ids=list(range(N_CORES)), trace=trace, **kw)
    return _assemble(res.results), res


def kernel(**inputs):
    out, _ = run(inputs, trace=False)
    return out


# revision 11
# speedup vs baseline: 1.6072x; 1.0040x over previous
"""Trainium2 Bass kernel for nn_CrossAttention_31078383354530.

Reference computation (b=2, n=m=2048, qd=1024, cd=768, heads=8, dh=128):
    q = x @ Wq; k = ctx @ Wk; v = ctx @ Wv  (split into 8 heads of 128)
    sim = (q @ k^T) * dh**-0.5 over the FLATTENED (b*n)=4096 token axis
    attn = softmax((sim - mean)*1.5 + mean) == softmax(1.5*scale*(q@k^T))
        exactly (the mean-centering is a per-row constant shift)
    out = attn @ v -> merge heads -> y = out @ Wout + bout

Sharding (8 cores): context-token-sharded K/V projection + AllGather of the
bf16 K/V (all heads), then each core runs all 8 heads' attention for its own
512-query-token slice and its own final projection -> the output is a
disjoint row-slice per core (no reduction needed on host).

All matmul operands are bf16 (host converts the f32 inputs), accumulation is
f32 in PSUM. Softmax rowsum is an ones-stationary matmul (M=1) fused into
the PV stream; normalization (reciprocal + gpsimd partition_broadcast +
tensor_tensor multiply) runs off the critical path on SBUF copies so PSUM
banks recycle quickly and the PE stays dense (HAM stays at full clock).
"""

import sys

if "/opt/trn_rl_repo" not in sys.path:
    sys.path.insert(0, "/opt/trn_rl_repo")

import ml_dtypes
import numpy as np

import concourse.bass as bass  # noqa: F401
import concourse.mybir as mybir
import concourse.tile as tile
from concourse import bacc, bass_utils

F32 = mybir.dt.float32
BF16 = mybir.dt.bfloat16
AF = mybir.ActivationFunctionType

P = 128
N_CORES = 8
HEADS = 8
DH = 128
TOK = 4096             # b*n flattened token axis (attention mixes batches!)
SLICE = TOK // N_CORES  # 512 tokens per core
QD = 1024
CD = 768
INNER = 1024
KC = QD // P           # 8 qd chunks
CC = CD // P           # 6 cd chunks
JT = TOK // P          # 32 j-tiles per head
GRP = 3                # j-tiles per exp group ([128, 1536] psum, 3 banks)
TAU_SCALE = 1.5 * (DH ** -0.5)

_CACHE = {}


def _build():
    nc = bacc.Bacc(num_devices=N_CORES)

    xTs = nc.declare_dram_parameter("xTs", [QD, SLICE], BF16, isOutput=False)
    cTs = nc.declare_dram_parameter("cTs", [CD, SLICE], BF16, isOutput=False)
    Wq = nc.declare_dram_parameter("Wq", [QD, INNER], BF16, isOutput=False)
    Wk = nc.declare_dram_parameter("Wk", [CD, INNER], BF16, isOutput=False)
    Wv = nc.declare_dram_parameter("Wv", [CD, INNER], BF16, isOutput=False)
    Wout = nc.declare_dram_parameter("Wout", [INNER, QD], BF16, isOutput=False)
    boutT = nc.declare_dram_parameter("boutT", [P, KC], F32, isOutput=False)
    yT = nc.declare_dram_parameter("yT", [KC, P, SLICE], F32, isOutput=True)

    with tile.TileContext(nc) as tc:
        with (
            tc.tile_pool(name="const", bufs=1) as const,
            tc.tile_pool(name="sb", bufs=1) as sb,
            tc.tile_pool(name="ps", bufs=1, space="PSUM") as ps,
            tc.tile_pool(name="dram", bufs=1, space="DRAM") as dram,
        ):
            # per-head-pair bounce buffers -> 4 pipelined AllGathers, so the
            # attention on early heads overlaps the later transfers
            NP = HEADS // 2
            kv_in = [dram.tile([2, 2, P, SLICE], BF16, name=f"kv_in{p}")
                     for p in range(NP)]
            kv_g = [dram.tile([N_CORES, 2, 2, P, SLICE], BF16,
                              addr_space="Shared", name=f"kv_g{p}")
                    for p in range(NP)]

            ones_b = const.tile([P, 1], BF16, name="ones_b")
            nc.vector.memset(ones_b[:], 1.0)
            bout_sb = const.tile([P, KC], F32, name="bout_sb")
            nc.sync.dma_start(bout_sb[:], boutT[:, :])

            # ---- K/V projection inputs: contiguous row-chunk tiles ----
            cts = []
            for k in range(CC):
                t = sb.tile([P, SLICE], BF16, name=f"cts{k}", tag="cts", bufs=CC)
                nc.sync.dma_start(t[:], cTs[k * P:(k + 1) * P, :])
                cts.append(t)
            wkt = []
            for k in range(CC):
                t = sb.tile([P, INNER], BF16, name=f"wkt{k}", tag="wkt", bufs=CC)
                nc.sync.dma_start(t[:], Wk[k * P:(k + 1) * P, :])
                wkt.append(t)
            wvt = []
            for k in range(CC):
                t = sb.tile([P, INNER], BF16, name=f"wvt{k}", tag="wvt", bufs=CC)
                nc.sync.dma_start(t[:], Wv[k * P:(k + 1) * P, :])
                wvt.append(t)

            # ---- K projection: kT (head-major, [d, tok]) for this ctx slice
            # psum tag "sim" is shared with the attention phase (temporal reuse)
            for m in range(HEADS):
                kps = ps.tile([P, GRP * SLICE], F32, name=f"kps{m}", tag="sim",
                              bufs=2)
                for k in range(CC):
                    nc.tensor.matmul(kps[:, :SLICE], wkt[k][:, m * DH:(m + 1) * DH],
                                     cts[k][:],
                                     start=(k == 0), stop=(k == CC - 1))
                ksb = sb.tile([P, SLICE], BF16, name=f"ksb{m}", tag="ksb", bufs=3)
                nc.vector.tensor_copy(ksb[:], kps[:, :SLICE])
                nc.sync.dma_start(kv_in[m // 2][0, m % 2], ksb[:])

            # ---- V projection: v (token-major, [tok, inner]) for this slice
            # half-major so heads 0..3 (pairs 0,1) are ready before heads 4..7
            vsb = [sb.tile([P, INNER], BF16, name=f"vsb{tt}", tag="vsb", bufs=4)
                   for tt in range(SLICE // P)]
            for half in range(2):
                for tt in range(SLICE // P):  # 4 token tiles
                    vps = ps.tile([P, GRP * SLICE], F32, name=f"vps{tt}_{half}",
                                  tag="sim", bufs=2)
                    for k in range(CC):
                        nc.tensor.matmul(
                            vps[:, :512],
                            cts[k][:, tt * P:(tt + 1) * P],
                            wvt[k][:, half * 512:(half + 1) * 512],
                            start=(k == 0), stop=(k == CC - 1))
                    nc.vector.tensor_copy(vsb[tt][:, half * 512:(half + 1) * 512],
                                          vps[:, :512])
                for h in range(4 * half, 4 * half + 4):
                    for tt in range(SLICE // P):
                        nc.sync.dma_start(
                            kv_in[h // 2][1, h % 2, :, tt * DH:(tt + 1) * DH],
                            vsb[tt][:, h * DH:(h + 1) * DH])
                # ---- pipelined AllGathers (2 heads per collective) ----
                for p in (2 * half, 2 * half + 1):
                    nc.gpsimd.collective_compute(
                        "AllGather", mybir.AluOpType.bypass,
                        replica_groups=[list(range(N_CORES))],
                        ins=[kv_in[p].opt()], outs=[kv_g[p].opt()],
                    )

            # ---- Q projection (all heads, own token slice); overlaps AG ----
            xts = []
            for k in range(KC):
                t = sb.tile([P, SLICE], BF16, name=f"xts{k}", tag="xts", bufs=KC)
                nc.sync.dma_start(t[:], xTs[k * P:(k + 1) * P, :])
                xts.append(t)
            wqt = []
            for k in range(KC):
                t = sb.tile([P, INNER], BF16, name=f"wqt{k}", tag="wqt", bufs=KC)
                nc.sync.dma_start(t[:], Wq[k * P:(k + 1) * P, :])
                wqt.append(t)
            qsb = []
            for m in range(HEADS):
                qps = ps.tile([P, GRP * SLICE], F32, name=f"qps{m}", tag="sim",
                              bufs=2)
                for k in range(KC):
                    nc.tensor.matmul(qps[:, :SLICE], wqt[k][:, m * DH:(m + 1) * DH],
                                     xts[k][:],
                                     start=(k == 0), stop=(k == KC - 1))
                qt = sb.tile([P, SLICE], BF16, name=f"qsb{m}", tag="qsb",
                             bufs=HEADS)
                nc.vector.tensor_copy(qt[:], qps[:, :SLICE])
                qsb.append(qt)

            # ---- attention, one head at a time over the full 4096 ctx ----
            groups = []
            j0 = 0
            while j0 < JT:
                groups.append(list(range(j0, min(j0 + GRP, JT))))
                j0 += GRP

            osb = [None] * HEADS
            for h in range(HEADS):
                kh = sb.tile([P, TOK], BF16, name=f"kh{h}", tag="kh", bufs=3)
                vh = sb.tile([P, TOK], BF16, name=f"vh{h}", tag="vh", bufs=3)
                for r in range(N_CORES):
                    nc.sync.dma_start(kh[:, r * SLICE:(r + 1) * SLICE],
                                      kv_g[h // 2][r, 0, h % 2])
                    nc.sync.dma_start(vh[:, r * SLICE:(r + 1) * SLICE],
                                      kv_g[h // 2][r, 1, h % 2])
                pv_ps = ps.tile([P, SLICE], F32, name=f"pv{h}", tag="pv", bufs=1)
                rs_ps = ps.tile([1, SLICE], F32, name=f"rs{h}", tag="rs", bufs=1)
                for g, js in enumerate(groups):
                    sim_ps = ps.tile([P, GRP * SLICE], F32, name=f"sim{h}_{g}",
                                     tag="sim", bufs=2)
                    for jj, j in enumerate(js):
                        nc.tensor.matmul(
                            sim_ps[:, jj * SLICE:(jj + 1) * SLICE],
                            kh[:, j * P:(j + 1) * P], qsb[h][:],
                            start=True, stop=True)
                    at = sb.tile([P, GRP * SLICE], BF16, name=f"at{h}_{g}",
                                 tag="at", bufs=4)
                    nc.scalar.activation(at[:, :len(js) * SLICE],
                                         sim_ps[:, :len(js) * SLICE], AF.Exp,
                                         scale=TAU_SCALE)
                    for jj, j in enumerate(js):
                        nc.tensor.matmul(pv_ps[:], vh[:, j * P:(j + 1) * P],
                                         at[:, jj * SLICE:(jj + 1) * SLICE],
                                         start=(j == 0), stop=(j == JT - 1))
                        nc.tensor.matmul(rs_ps[:], ones_b[:],
                                         at[:, jj * SLICE:(jj + 1) * SLICE],
                                         start=(j == 0), stop=(j == JT - 1))
                # free the PSUM banks quickly; normalization runs off the
                # critical path on SBUF copies (overlaps the next head)
                pvc = sb.tile([P, SLICE], F32, name=f"pvsb{h}", tag="pvsb",
                              bufs=2)
                nc.vector.tensor_copy(pvc[:], pv_ps[:])
                rsc = sb.tile([1, SLICE], F32, name=f"rssb{h}", tag="rssb",
                              bufs=2)
                nc.vector.tensor_copy(rsc[:], rs_ps[:])
                recip = sb.tile([1, SLICE], F32, name=f"recip{h}", tag="recip",
                                bufs=2)
                nc.vector.reciprocal(recip[:], rsc[:])
                bc = sb.tile([P, SLICE], F32, name=f"bc{h}", tag="bc", bufs=2)
                nc.gpsimd.partition_broadcast(bc[:], recip[:])
                ot = sb.tile([P, SLICE], BF16, name=f"osb{h}", tag="osb",
                             bufs=HEADS)
                nc.vector.tensor_tensor(ot[:], pvc[:], bc[:],
                                        mybir.AluOpType.mult)
                osb[h] = ot

            # ---- final projection: yT[cc] = Wout[:, cc]^T @ out^T + bout ----
            for cc in range(KC):
                wo = sb.tile([P, KC, DH], BF16, name=f"wo{cc}", tag="wo", bufs=4)
                nc.sync.dma_start(
                    wo[:],
                    Wout.ap()[:, cc * DH:(cc + 1) * DH].rearrange(
                        "(k p) c -> p k c", p=P),
                )
                # alternate between the two single-bank psum tags so
                # consecutive output chunks double-buffer
                yps = ps.tile([P, SLICE], F32, name=f"yps{cc}",
                              tag=("pv" if cc % 2 == 0 else "rs"), bufs=1)
                for ic in range(HEADS):
                    nc.tensor.matmul(yps[:], wo[:, ic], osb[ic][:],
                                     start=(ic == 0), stop=(ic == HEADS - 1))
                yt = sb.tile([P, SLICE], F32, name=f"yt{cc}", tag="yt", bufs=2)
                nc.scalar.activation(yt[:], yps[:], AF.Identity,
                                     bias=bout_sb[:, cc:cc + 1], scale=1.0)
                nc.sync.dma_start(yT.ap()[cc], yt[:])

    nc.compile()
    return nc


def _get_nc():
    if "nc" not in _CACHE:
        _CACHE["nc"] = _build()
    return _CACHE["nc"]


def _bf16(a):
    return np.ascontiguousarray(np.asarray(a, np.float32).astype(ml_dtypes.bfloat16))


def _prep_in_maps(x, context, Wq, Wk, Wv, Wout, bout):
    x_f = np.asarray(x, dtype=np.float32).reshape(TOK, QD)
    c_f = np.asarray(context, dtype=np.float32).reshape(TOK, CD)
    Wq = _bf16(Wq)
    Wk = _bf16(Wk)
    Wv = _bf16(Wv)
    Wout = _bf16(Wout)
    boutT = np.ascontiguousarray(
        np.asarray(bout, dtype=np.float32).reshape(KC, P).T)
    in_maps = []
    for c in range(N_CORES):
        sl = slice(c * SLICE, (c + 1) * SLICE)
        in_maps.append({
            "xTs": _bf16(x_f[sl].T),
            "cTs": _bf16(c_f[sl].T),
            "Wq": Wq, "Wk": Wk, "Wv": Wv, "Wout": Wout, "boutT": boutT,
        })
    return in_maps


def _assemble(results):
    y = np.empty((TOK, QD), dtype=np.float32)
    for c in range(N_CORES):
        yt = results[c]["yT"]   # [KC, P, SLICE]
        y[c * SLICE:(c + 1) * SLICE] = (
            yt.transpose(2, 0, 1).reshape(SLICE, QD))
    return y.reshape(2, TOK // 2, QD)


def run(inputs, trace=False, **kw):
    nc = _get_nc()
    in_maps = _prep_in_maps(**inputs)
    res = bass_utils.run_bass_kernel_spmd(
        nc, in_maps, core_ids=list(range(N_CORES)), trace=trace, **kw)
    return _assemble(res.results), res


def kernel(**inputs):
    out, _ = run(inputs, trace=False)
    return out
